# revision 43
# baseline (speedup 1.0000x reference)
"""Additive-attention (Bahdanau) kernel for Trainium2, 8 NeuronCores.

Computes attns[b, n, m] = sum_h v[h] * tanh(hq[b, h, n] + hk[b, h, m])
where hq = Wq @ q[b], hk = Wk @ k[b], returned flattened as (B, NQ*NK).

Strategy (data-parallel over batch, 4 batches per core):
  - hq/hk via fp32 PE matmuls (host-pretransposed W as lhsT); hq kept
    fp32 (scalar operand), hk cast fp16.
  - preact[h, (n,m)] = hk + hq[:, n] built per-query with DVE
    tensor_scalar_add (fp16 streams at 2x mode, ~196ns per 128x256).
  - tanh on ScalarE in big fp16 instructions -- the bottleneck engine:
    ~16.8M tanh elems/core at 128 lanes @ 1.2 GHz ~= 114us busy.
  - v-contraction over h on PE: v half replicated to (128,32) stationary,
    fp16 tanh slab rhs N=512 per matmul, 2 h-halves accumulated in PSUM;
    4 query-pairs share each PSUM bank via col-tiling (tile_position) at
    partitions 0/32/64/96; two banks per PSUM tile.
  - PSUM->SBUF copy on DVE (deferred one unit to keep DVE streaming),
    strided DMA to HBM. Both DVE and ACT end ~120us busy; ~151us wall.
"""

import sys

sys.path.insert(0, "/opt/trn_rl_repo")

from contextlib import ExitStack

import numpy as np

import concourse.bacc as bacc
import concourse.bass as bass
import concourse.mybir as mybir
import concourse.tile as tile
from concourse.bass_utils import run_bass_kernel_spmd

B, HID, QH, KH, NQ, NK = 32, 256, 256, 256, 64, 256
NCORES = 8
BPC = B // NCORES  # batches per core
NCHUNK = 2  # query chunks per batch
QPC = NQ // NCHUNK  # queries per chunk (32)
PAIRS = QPC // 2  # query pairs per chunk (16)
GROUPS = PAIRS // 4  # groups of 4 pairs per chunk (4)

f32 = mybir.dt.float32
f16 = mybir.dt.float16

_NC_CACHE = {}


def build_nc():
    nc = bacc.Bacc("TRN2", target_bir_lowering=False, debug=False)

    q_d = nc.dram_tensor("q", [BPC, 2, 128, NQ], f16, kind="ExternalInput")
    k_d = nc.dram_tensor("k", [BPC, 2, 128, NK], f16, kind="ExternalInput")
    wqt_d = nc.dram_tensor("wqt", [2, 2, 128, HID], f16, kind="ExternalInput")
    wkt_d = nc.dram_tensor("wkt", [2, 2, 128, HID], f16, kind="ExternalInput")
    vh_d = nc.dram_tensor("vh", [128, 64], f16, kind="ExternalInput")
    out_d = nc.dram_tensor("out", [BPC, 2 * GROUPS, 4, 512], f32, kind="ExternalOutput")

    with tile.TileContext(nc) as tc, ExitStack() as ctx:
        wpool = ctx.enter_context(tc.tile_pool(name="wpool", bufs=1))
        iopool = ctx.enter_context(tc.tile_pool(name="iopool", bufs=3))
        hpool = ctx.enter_context(tc.tile_pool(name="hpool", bufs=4))
        prepool = ctx.enter_context(tc.tile_pool(name="prepool", bufs=3))
        tanhpool = ctx.enter_context(tc.tile_pool(name="tanhpool", bufs=5))
        obpool = ctx.enter_context(tc.tile_pool(name="obpool", bufs=6))
        psA = ctx.enter_context(tc.tile_pool(name="psA", bufs=2, space="PSUM"))
        psO = ctx.enter_context(tc.tile_pool(name="psO", bufs=3, space="PSUM"))

        # Preload the tanh ACT table at t=0 (overlaps with input DMAs).
        warm = wpool.tile([128, 2], f16, name="warm", tag="warm")
        nc.vector.memset(warm[:, 0:1], 0.0)
        nc.scalar.activation(
            warm[:, 1:2], warm[:, 0:1], mybir.ActivationFunctionType.Tanh
        )

        def load_qk(b, eng=None):
            eng = eng or nc.gpsimd
            q_sb = iopool.tile([128, 2 * NQ], f16, name=f"q_sb{b}", tag="qsb")
            k_sb = iopool.tile([128, 2 * NK], f16, name=f"k_sb{b}", tag="ksb")
            eng.dma_start(
                q_sb[:].rearrange("p (kb n) -> p kb n", kb=2),
                q_d[b].rearrange("kb p n -> p kb n"),
            )
            eng.dma_start(
                k_sb[:].rearrange("p (kb n) -> p kb n", kb=2),
                k_d[b].rearrange("kb p n -> p kb n"),
            )
            return q_sb, k_sb

        q0_sb = iopool.tile([128, 2 * NQ], f16, name="q_sb0", tag="qsb")
        k0_sb = iopool.tile([128, 2 * NK], f16, name="k_sb0", tag="ksb")
        wq_sb = []
        wk_sb = []
        for kb in range(2):
            wq_t = wpool.tile([128, 2 * HID], f16, name=f"wq_sb{kb}", tag=f"wq{kb}")
            wq_sb.append(wq_t)
            wk_t = wpool.tile([128, 2 * HID], f16, name=f"wk_sb{kb}", tag=f"wk{kb}")
            wk_sb.append(wk_t)
        vh_sb = wpool.tile([128, 64], f16, name="vh_sb", tag="vh")
        # Critical startup DMAs issue from gpsimd (its preamble finishes
        # ~3us before sync's), in the exact order the first matmuls need.
        nc.gpsimd.dma_start(
            q0_sb[:].rearrange("p (kb n) -> p kb n", kb=2),
            q_d[0].rearrange("kb p n -> p kb n"),
        )
        nc.gpsimd.dma_start(
            wq_sb[0][:].rearrange("p (t h) -> p t h", t=2),
            wqt_d[0].rearrange("t p h -> p t h"),
        )
        nc.gpsimd.dma_start(
            wq_sb[1][:].rearrange("p (t h) -> p t h", t=2),
            wqt_d[1].rearrange("t p h -> p t h"),
        )
        nc.gpsimd.dma_start(
            k0_sb[:].rearrange("p (kb n) -> p kb n", kb=2),
            k_d[0].rearrange("kb p n -> p kb n"),
        )
        nc.scalar.dma_start(
            wk_sb[0][:].rearrange("p (t h) -> p t h", t=2),
            wkt_d[0].rearrange("t p h -> p t h"),
        )
        nc.scalar.dma_start(
            wk_sb[1][:].rearrange("p (t h) -> p t h", t=2),
            wkt_d[1].rearrange("t p h -> p t h"),
        )
        nc.scalar.dma_start(vh_sb[:], vh_d[:])
        qk = {0: (q0_sb, k0_sb)}
        hqhk = {}

        def make_hqhk(b):
            # b0 casts gate the first adds -> DVE (free then); later batches'
            # casts slot into ACT's ramp-idle gaps instead of costing DVE.
            cast = nc.vector.tensor_copy if b == 0 else nc.scalar.copy
            q_sb, k_sb = qk.pop(b)
            hq32 = hpool.tile([128, 2 * NQ], f32, name=f"hq32_{b}", tag="hq32")
            hk16 = hpool.tile([128, 2 * NK], f16, name=f"hk16_{b}", tag="hk16")
            nt = 1 if b == 0 else 2  # b0: hi-only W, halves the cold start chain
            for j in range(2):
                ps_hq = psA.tile([128, NQ], f32, name=f"ps_hq{b}_{j}", tag="psA")
                for kb in range(2):
                    for t in range(nt):  # W = hi + lo fp16 split
                        nc.tensor.matmul(
                            ps_hq[:],
                            wq_sb[kb][:, t * HID + 128 * j : t * HID + 128 * (j + 1)],
                            q_sb[:, bass.ts(kb, NQ)],
                            start=(kb == 0 and t == 0),
                            stop=(kb == 1 and t == nt - 1),
                        )
                cast(hq32[:, bass.ts(j, NQ)], ps_hq[:])
                ps_hk = psA.tile([128, NK], f32, name=f"ps_hk{b}_{j}", tag="psA")
                for kb in range(2):
                    for t in range(nt):
                        nc.tensor.matmul(
                            ps_hk[:],
                            wk_sb[kb][:, t * HID + 128 * j : t * HID + 128 * (j + 1)],
                            k_sb[:, bass.ts(kb, NK)],
                            start=(kb == 0 and t == 0),
                            stop=(kb == 1 and t == nt - 1),
                        )
                cast(hk16[:, bass.ts(j, NK)], ps_hk[:])
            hqhk[b] = (hq32, hk16)

        make_hqhk(0)
        qk[1] = load_qk(1)
        make_hqhk(1)
        qk[2] = load_qk(2)

        # Work units: (batch, qlo, nq). Fine-grained at the start so ACT
        # ramps early, 16-query pieces at the end for a short drain; full
        # 32-query chunks in steady state.
        units = []
        for b in range(BPC):
            if b == 0:
                units += [(0, 0, 8), (0, 8, 8), (0, 16, 16), (0, 32, 16), (0, 48, 16)]
            elif b == BPC - 1:
                units += [(b, 0, 32), (b, 32, 16), (b, 48, 8), (b, 56, 8)]
            else:
                units += [(b, 0, 32), (b, 32, 32)]

        deferred = []
        for ui, (b, qlo, nq) in enumerate(units):
            hq32, hk16 = hqhk[b]
            if ui == 0:
                qk[3] = load_qk(3)
            elif ui == 1:
                make_hqhk(2)
            elif ui == 2:
                make_hqhk(3)

            th = []
            for j in range(2):
                pre = prepool.tile(
                    [128, nq * NK], f16, name=f"pre{b}_{qlo}_{j}", tag="pre"
                )
                for nn in range(nq):
                    n = qlo + nn
                    nc.vector.tensor_scalar_add(
                        pre[:, bass.ts(nn, NK)],
                        hk16[:, bass.ts(j, NK)],
                        hq32[:, j * NQ + n : j * NQ + n + 1],
                    )
                t_ = tanhpool.tile(
                    [128, nq * NK], f16, name=f"tanh{b}_{qlo}_{j}", tag="tanh"
                )
                if b == 0 and qlo == 0:
                    half = nq * NK // 2
                    nc.scalar.activation(
                        t_[:, :half], pre[:, :half], mybir.ActivationFunctionType.Tanh
                    )
                    nc.scalar.activation(
                        t_[:, half:], pre[:, half:], mybir.ActivationFunctionType.Tanh
                    )
                else:
                    nc.scalar.activation(
                        t_[:], pre[:], mybir.ActivationFunctionType.Tanh
                    )
                th.append(t_)
                if j == 0:
                    for bb, gg, w, pss in deferred:
                        ob = obpool.tile(
                            [128, 512 * w], f32, name=f"ob{bb}_{gg}", tag="ob"
                        )
                        if gg == 4 and bb == 1:
                            nc.scalar.copy(ob[:], pss[:])
                        else:
                            nc.vector.tensor_copy(ob[:], pss[:])
                        dst = out_d[bb, gg : gg + w].rearrange("g r c -> r g c")
                        srcap = ob[0:128:32, :].rearrange("p (g c) -> p g c", g=w)
                        nc.sync.dma_start(dst, srcap)
                    deferred = []

            tails = []
            ngroups = nq // 8
            g = 0
            while g < ngroups:
                w = 2 if ngroups - g >= 2 else 1  # banks per psum tile
                ps = psO.tile(
                    [128, 512 * w], f32, name=f"ps{b}_{qlo}_{g}", tag="psO"
                )
                for gg in range(w):
                    for j in range(2):
                        for r in range(4):
                            p = 4 * (g + gg) + r
                            nc.tensor.matmul(
                                ps[32 * r : 32 * r + 32, bass.ts(gg, 512)],
                                vh_sb[:, bass.ts(j, 32)],
                                th[j][:, bass.ts(p, 512)],
                                start=(j == 0),
                                stop=(j == 1),
                                tile_position=(0, 32 * r),
                                skip_group_check=True,
                            )
                tails.append((b, qlo // 8 + g, w, ps))
                g += w

            deferred = tails

        for i, (bb, gg, w, pss) in enumerate(deferred):
            ob = obpool.tile([128, 512 * w], f32, name=f"ob{bb}_{gg}", tag="ob")
            nc.vector.tensor_copy(ob[:], pss[:])
            dst = out_d[bb, gg : gg + w].rearrange("g r c -> r g c")
            srcap = ob[0:128:32, :].rearrange("p (g c) -> p g c", g=w)
            nc.sync.dma_start(dst, srcap)

    nc.compile()
    return nc


def get_nc():
    if "nc" not in _NC_CACHE:
        _NC_CACHE["nc"] = build_nc()
    return _NC_CACHE["nc"]


def make_in_maps(att_query, att_key, v, W):
    att_query = np.ascontiguousarray(np.asarray(att_query, dtype=np.float32))
    att_key = np.ascontiguousarray(np.asarray(att_key, dtype=np.float32))
    v = np.asarray(v, dtype=np.float32)
    W = np.asarray(W, dtype=np.float32)

    q_all = att_query.astype(np.float16).reshape(NCORES, BPC, 2, 128, NQ)
    k_all = att_key.astype(np.float16).reshape(NCORES, BPC, 2, 128, NK)
    WqT = W[:, :QH].T  # (QH, H) fp32
    WkT = W[:, QH:].T
    def hilo(M):
        hi = M.astype(np.float16)
        lo = (M - hi.astype(np.float32)).astype(np.float16)
        # (2kb, 2t, 128, H): kb = contraction row block, t = hi/lo
        return np.ascontiguousarray(
            np.stack([hi.reshape(2, 128, HID), lo.reshape(2, 128, HID)], axis=1)
        )
    wqt = hilo(WqT)
    wkt = hilo(WkT)
    vh = np.ascontiguousarray(np.repeat(v.astype(np.float16).reshape(2, 128).T, 32, axis=1))

    return [
        {
            "q": np.ascontiguousarray(q_all[c]),
            "k": np.ascontiguousarray(k_all[c]),
            "wqt": wqt,
            "wkt": wkt,
            "vh": vh,
        }
        for c in range(NCORES)
    ]


def _ensure_ntff_hook():
    """Register the axon NTFF profile hook (image's antenv lacks axon_hooks)."""
    import types

    try:
        import antenv.axon_hooks  # noqa: F401
    except ImportError:
        import antenv

        mod = types.ModuleType("antenv.axon_hooks")
        _hook = [None]
        mod.set_axon_ntff_profile_hook = lambda h: _hook.__setitem__(0, h)
        mod.get_axon_ntff_profile_hook = lambda: _hook[0]
        sys.modules["antenv.axon_hooks"] = mod
        antenv.axon_hooks = mod
    from antenv.axon_hooks import (
        get_axon_ntff_profile_hook,
        set_axon_ntff_profile_hook,
    )

    if get_axon_ntff_profile_hook() is None:
        from trn_agent_boot.trn_boot import _ntff_profile_via_ctypes

        set_axon_ntff_profile_hook(_ntff_profile_via_ctypes("/opt/axon/libaxon_pjrt.so"))


def run(att_query, att_key, v, W, trace=False, **kwargs):
    nc = get_nc()
    if trace:
        _ensure_ntff_hook()
    in_maps = make_in_maps(att_query, att_key, v, W)
    res = run_bass_kernel_spmd(
        nc, in_maps, core_ids=list(range(NCORES)), trace=trace, **kwargs
    )
    outs = [np.asarray(res.results[c]["out"]).reshape(BPC, NQ * NK) for c in range(NCORES)]
    return np.concatenate(outs, axis=0), res


def kernel(att_query, att_key, v, W):
    out, _ = run(att_query, att_key, v, W)
    return out
